# revision 26
# baseline (speedup 1.0000x reference)
"""Multi-head attention (B=16, N=1024, D=1024, H=8, dh=128) on 8 trn2 cores.

Strategy: data-parallel over batch (2 batches/core), bf16 matmuls (fp32 PSUM
accumulation; end-to-end rel err ~4e-3 vs the 2e-2 gate).

Layout / schedule:
  * All four weight matrices are cast to bf16 on host and DMA'd ONCE at
    kernel start into resident SBUF tiles (16KB/partition each), issue order
    Wq -> x(b0) -> Wk -> Wv -> Wo -> x(b1) so the first projection unit's
    inputs land first (the startup is DMA-bandwidth-bound).
  * Per batch, per 2-head group g: Q^T_g, K^T_g (head-transposed: dh on
    partitions) and V_g (natural) via bf16 matmuls from x^T (host-side
    pre-transposed + pre-cast) and the resident weight tiles.
  * Per head, per 512-wide q chunk: S^T = K_h^T.T @ Q_h^T (k on partitions),
    E^T = exp(norm*S^T) on ACT (bf16 out), heads^T += V_h.T @ E^T, and
    R = colsum(E^T) via DVE pairwise adds plus one all-ones 128x128 matmul
    that yields R broadcast to every partition; 1/R via a fast reciprocal,
    applied while writing heads^T to SBUF.
  * Phase 3: out = heads_norm @ Wo in natural layout; the resident Wo makes
    this DMA-free. Attention units are queued qc-major so the LAST batch's
    phase 3 for rows 0:512 can interleave with the remaining qc=1 attention
    units, keeping PE busy through the tail.
  * Output is written bf16 and upcast on host.
"""

import numpy as np
import ml_dtypes

import concourse.bass as bass
import concourse.mybir as mybir
import concourse.tile as tile
from concourse import bacc
from concourse.bass_utils import run_bass_kernel_spmd

N_CORES = 8
B = 16
BPC = B // N_CORES      # batches per core
N = 1024                # sequence length
D = 1024                # model dim
H = 8                   # heads
DH = 128                # head dim
P = 128
DB = D // P             # 8 contraction blocks
GH = 2                  # heads per group
G = H // GH             # 4 groups
GW = GH * DH            # 256: e-width per group
NC2 = N // 512          # 2 n-chunks of 512
NORM = 1.0 / np.sqrt(DH)

F32 = mybir.dt.float32
BF16 = mybir.dt.bfloat16


def build_nc(has_bias=True):
    """All big DRAM tensors are host-side packed partition-major so each
    DMA moves long contiguous per-partition runs (4-16KB descriptors):
      xT  [BPC, P, DB, N]   xT[b,p,db,n]  = x[b, n, db*128+p]
      Wq/Wk/Wv [P, G, DB, GW]  W[p,g,db,e] = W[db*128+p, g*256+e]
      Wo  [P, DB, D]        Wo[p,db,e]    = Wo[db*128+p, e]
      out [BPC, P, DB, D]   out[b,p,nb,d] = out_full[b, nb*128+p, d]
    """
    nc = bacc.Bacc()
    xT = nc.declare_dram_parameter("xT", [BPC, P, DB, N], BF16, isOutput=False)
    Wq = nc.declare_dram_parameter("Wq", [P, G, DB, GW], BF16, isOutput=False)
    Wk = nc.declare_dram_parameter("Wk", [P, G, DB, GW], BF16, isOutput=False)
    Wv = nc.declare_dram_parameter("Wv", [P, G, DB, GW], BF16, isOutput=False)
    Wo = nc.declare_dram_parameter("Wo", [P, DB, D], BF16, isOutput=False)
    bq = nc.declare_dram_parameter("bq", [D], F32, isOutput=False)
    bk = nc.declare_dram_parameter("bk", [D], F32, isOutput=False)
    bv = nc.declare_dram_parameter("bv", [D], BF16, isOutput=False)
    out = nc.declare_dram_parameter("out", [BPC, P, DB, D], BF16, isOutput=True)

    with tile.TileContext(nc) as tc:
        with tc.tile_pool(name="wres", bufs=1) as wres, \
             tc.tile_pool(name="big", bufs=1) as big, \
             tc.tile_pool(name="work", bufs=1) as work, \
             tc.tile_pool(name="small", bufs=1) as small, \
             tc.tile_pool(name="ps", bufs=1, space="PSUM") as ps:

            # constants / biases
            bq_col = small.tile([P, DB], F32, name="bq_col")
            bk_col = small.tile([P, DB], F32, name="bk_col")
            bv_col = small.tile([P, DB], BF16, name="bv_col")
            if has_bias:
                nc.sync.dma_start(out=bq_col, in_=bq.rearrange("(eb p) -> p eb", p=P))
                nc.sync.dma_start(out=bk_col, in_=bk.rearrange("(eb p) -> p eb", p=P))
                nc.sync.dma_start(out=bv_col, in_=bv.rearrange("(eb p) -> p eb", p=P))
            ones128_f32 = small.tile([P, P], F32, name="ones128_f32")
            nc.vector.memset(ones128_f32, 1.0)
            ones128 = small.tile([P, P], BF16, name="ones128")
            nc.vector.tensor_copy(ones128, ones128_f32)
            ones_row_f32 = small.tile([1, P], F32, name="ones_row_f32")
            nc.vector.memset(ones_row_f32, 1.0)
            ones_row = small.tile([1, P], BF16, name="ones_row")
            nc.vector.tensor_copy(ones_row, ones_row_f32)
            c_sb = small.tile([1, NC2, 512], BF16, name="c_sb")

            # ---- resident weights + x, issued in first-needed order.
            # Weights go on the Sync HWDGE ring, x + output stores on the
            # Scalar ring: the two rings issue concurrently (DMA issue is
            # ~0.6-2us per dma_start, serial per ring) and the 16 SDMA
            # engines round-robin between them at packet granularity.
            wq_r = wres.tile([P, G, DB, GW], BF16, name="wq_r")
            wk_r = wres.tile([P, G, DB, GW], BF16, name="wk_r")
            wv_r = wres.tile([P, G, DB, GW], BF16, name="wv_r")
            wo_r = wres.tile([P, DB, D], BF16, name="wo_r")

            def load_xt(b):
                # split along N so the first (nch=0) projection units only
                # need the first half of the transfer
                xt = big.tile([P, DB, N], BF16, name=f"xt_b{b}", tag="xt", bufs=2)
                for h2 in range(2):
                    nc.scalar.dma_start(
                        out=xt[:, :, h2 * 512:(h2 + 1) * 512],
                        in_=xT[b][:, :, h2 * 512:(h2 + 1) * 512])
                return xt

            nc.sync.dma_start(out=wq_r[:, 0], in_=Wq[:, 0])   # first needed
            xts = [load_xt(0)]
            nc.sync.dma_start(out=wk_r[:, 0], in_=Wk[:, 0])
            nc.sync.dma_start(out=wv_r[:, 0], in_=Wv[:, 0])
            # bulk loads: issue is gated (add_dep_helper below) on the first
            # projection unit so their packets don't steal DMA bandwidth
            # from the startup-critical wq[g0] + x(b0) transfers
            rest_dmas = [
                nc.sync.dma_start(out=wq_r[:, 1:], in_=Wq[:, 1:]),
                nc.sync.dma_start(out=wk_r[:, 1:], in_=Wk[:, 1:]),
                nc.sync.dma_start(out=wv_r[:, 1:], in_=Wv[:, 1:]),
                nc.sync.dma_start(out=wo_r, in_=Wo[:, :, :]),
            ]

            def emit_proj_unit(b, g, kind, idx, xt, qTg, kTg, vg):
                """Emit one psum accumulation group of phase 1."""
                gsfx = f"_b{b}_g{g}"
                e0 = g * GW
                if kind in ("q", "k"):
                    dst, wt, bcol = ((qTg, wq_r, bq_col) if kind == "q"
                                     else (kTg, wk_r, bk_col))
                    eb, nch = divmod(idx, NC2)
                    acc = ps.tile([P, 512], F32, tag="pj", bufs=2,
                                  name=f"p{kind}{gsfx}_{eb}_{nch}")
                    for db in range(DB):
                        nc.tensor.matmul(
                            acc,
                            wt[:, g, db, eb * P:(eb + 1) * P],
                            xt[:, db, nch * 512:(nch + 1) * 512],
                            start=(db == 0), stop=(db == DB - 1))
                    ebg = (e0 // P) + eb
                    if has_bias:
                        cp = nc.vector.tensor_scalar_add(
                            dst[:, eb, nch * 512:(nch + 1) * 512],
                            acc, bcol[:, ebg:ebg + 1])
                    else:
                        cp = nc.vector.tensor_copy(
                            dst[:, eb, nch * 512:(nch + 1) * 512], acc)
                    return cp
                else:  # "v"
                    nb = idx
                    accv = ps.tile([P, 512], F32, tag="pj", bufs=2,
                                   name=f"pv{gsfx}_{nb}")
                    for db in range(DB):
                        nc.tensor.matmul(
                            accv[:, :GW],
                            xt[:, db, nb * P:(nb + 1) * P],
                            wv_r[:, g, db, :],
                            start=(db == 0), stop=(db == DB - 1))
                    nc.vector.tensor_copy(vg[:, nb, :], accv[:, :GW])

            def make_phase3_half(b, hT, half):
                """Output projection for q rows [half*512, (half+1)*512).

                Pops one queued attention unit after every 3rd po unit so
                the po matmuls hide the attention units' exp/DVE waits.
                """
                def emit():
                    sfx = f"_b{b}"
                    if b == 0 and has_bias and half == 0:
                        # c = bv @ Wo (once; reused for b=1)
                        for oc in range(NC2):
                            pc = ps.tile([1, 512], F32, tag="pj", bufs=2,
                                         name=f"pc_{oc}")
                            for eb in range(DB):
                                nc.tensor.matmul(pc, bv_col[:, eb:eb + 1],
                                                 wo_r[:, eb, oc * 512:(oc + 1) * 512],
                                                 start=(eb == 0), stop=(eb == DB - 1))
                            nc.vector.tensor_copy(c_sb[:, oc, :], pc)
                    pi = 0
                    for nb in range(half * (DB // 2), (half + 1) * (DB // 2)):
                        osb = work.tile([P, D], BF16, tag="osb", bufs=2,
                                        name=f"o{sfx}_{nb}")
                        for oc in range(NC2):
                            po = ps.tile([P, 512], F32, tag="pj", bufs=2,
                                         name=f"po{sfx}_{oc}_{nb}")
                            for eb in range(H):
                                nc.tensor.matmul(
                                    po,
                                    hT[:, eb, nb * P:(nb + 1) * P],
                                    wo_r[:, eb, oc * 512:(oc + 1) * 512],
                                    start=(eb == 0),
                                    stop=(not has_bias and eb == H - 1))
                            if has_bias:
                                nc.tensor.matmul(po, ones_row, c_sb[:, oc, :],
                                                 start=False, stop=True)
                            nc.vector.tensor_copy(
                                osb[:, oc * 512:(oc + 1) * 512], po)
                            pi += 1
                            if pi % 2 == 0:
                                if pi % 4 == 2:
                                    pop_a()
                                else:
                                    pop_b()
                        nc.scalar.dma_start(out=out[b, :, nb, :], in_=osb)
                return emit

            # attention units and the previous batch's output projection are
            # emitted interleaved with later projection units so PE always
            # has ready matmuls during exp/epilogue waits
            attn_queue = []   # (emitA, emitB) pairs not yet started
            b_queue = []      # emitB parts whose emitA already ran

            def pop_a():
                if attn_queue:
                    a, bb = attn_queue.pop(0)
                    a()
                    b_queue.append(bb)

            def pop_b():
                if b_queue:
                    b_queue.pop(0)()

            pending = []   # deferred emit thunks (prev batch phase3 halves)

            for b in range(BPC):
                sfx = f"_b{b}"
                xt = xts[b]
                hT = None

                for g in range(G):
                    if b == 0 and g == 2:
                        # x(b1) load: emitted here so its issue lands after
                        # g0/g1's exps on the Scalar queue — keeps its
                        # packets out of the startup window
                        xts.append(load_xt(1))
                    gsfx = f"{sfx}_g{g}"
                    qTg = work.tile([P, GH, N], BF16, name=f"qT{gsfx}", tag="qTg", bufs=2)
                    kTg = work.tile([P, GH, N], BF16, name=f"kT{gsfx}", tag="kTg", bufs=2)
                    vg = work.tile([P, DB, GW], BF16, name=f"v{gsfx}", tag="vg", bufs=2)

                    # 16 proj units: 4 Q, 4 K, 8 V, ordered nch-major (all
                    # n<512 work first) so the startup only waits on the
                    # first half of the x transfer; interleave with pending
                    # attention unit parts (1 part per 2 proj units)
                    units = []
                    for nch in range(NC2):
                        units += ([("q", eb * NC2 + nch) for eb in range(GH)]
                                  + [("k", eb * NC2 + nch) for eb in range(GH)]
                                  + [("v", nch * (DB // 2) + i)
                                     for i in range(DB // 2)])
                    for ui, (kind, idx) in enumerate(units):
                        cp = emit_proj_unit(b, g, kind, idx, xt, qTg, kTg, vg)
                        if b == 0 and g == 0 and ui == 0:
                            # release the bulk weight loads once the first
                            # unit has run (startup-critical DMAs done)
                            for dma in rest_dmas:
                                tile.add_dep_helper(dma.ins, cp.ins, sync=True,
                                                    reason="delay bulk loads")
                        if ui % 4 == 1:
                            pop_a()
                        elif ui % 4 == 3:
                            pop_b()
                    if hT is None:
                        hT = big.tile([P, H, N], BF16, name=f"hT{sfx}", tag="hT", bufs=2)

                    # ---- queue attention for the heads of this group.
                    # Each unit is split into two emission parts: A does the
                    # PE-heavy scores/exp/AV plus the DVE row-folds; B does
                    # the colsum matmul + reciprocal + normalize. B is queued
                    # a couple of matmul-units after A so the colsum matmul
                    # never makes the in-order PE queue wait on DVE.
                    def make_attn(g, hh, qc, qTg=qTg, kTg=kTg, vg=vg, hT=hT, b=b):
                        h = g * GH + hh
                        asfx = f"_b{b}_h{h}_q{qc}"
                        state = {}

                        def emitA():
                            eT = work.tile([P, 4, 1024], BF16, name=f"eT{asfx}",
                                           tag="eT", bufs=2)
                            for j in range(4):
                                # scores for kb=2j, 2j+1 into one 2-bank tile
                                sp = ps.tile([P, 1024], F32, tag="spair", bufs=2,
                                             name=f"sp{asfx}_{j}")
                                for half in range(2):
                                    kb = 2 * j + half
                                    nc.tensor.matmul(
                                        sp[:, half * 512:(half + 1) * 512],
                                        kTg[:, hh, kb * P:(kb + 1) * P],
                                        qTg[:, hh, qc * 512:(qc + 1) * 512],
                                        start=True, stop=True)
                                nc.scalar.activation(
                                    eT[:, j, :], sp,
                                    mybir.ActivationFunctionType.Exp,
                                    scale=float(NORM))

                            # heads^T (unnormalized): [dv(128) x q(512)]
                            pav = ps.tile([P, 512], F32, tag="pav", bufs=1,
                                          name=f"pav{asfx}")
                            for j in range(4):
                                for half in range(2):
                                    kb = 2 * j + half
                                    nc.tensor.matmul(
                                        pav,
                                        vg[:, kb, hh * DH:(hh + 1) * DH],
                                        eT[:, j, half * 512:(half + 1) * 512],
                                        start=(kb == 0), stop=(kb == DB - 1))

                            # R = col-sum of E^T: pairwise adds on DVE
                            add = mybir.AluOpType.add
                            tA = work.tile([P, 1024], BF16, name=f"tA{asfx}", tag="tA", bufs=1)
                            tB = work.tile([P, 1024], BF16, name=f"tB{asfx}", tag="tB", bufs=1)
                            rp = work.tile([P, 512], BF16, name=f"rp{asfx}", tag="rp", bufs=2)
                            nc.vector.tensor_tensor(tA, eT[:, 0, :], eT[:, 1, :], add)
                            nc.vector.tensor_tensor(tB, eT[:, 2, :], eT[:, 3, :], add)
                            nc.vector.tensor_tensor(tA, tA, tB, add)
                            nc.vector.tensor_tensor(rp, tA[:, 0:512], tA[:, 512:1024], add)
                            state["pav"], state["rp"] = pav, rp

                        def emitB():
                            pav, rp = state["pav"], state["rp"]
                            # colsum of rp, broadcast to all partitions, in
                            # one matmul: every row of ones128.T @ rp is R
                            pbc = ps.tile([P, 512], F32, tag="pnorm", bufs=1, name=f"pbc{asfx}")
                            nc.tensor.matmul(pbc, ones128, rp,
                                             start=True, stop=True)
                            # 1/R at full 128-lane width (approx + one NR pass)
                            scratch = work.tile([P, 512], F32, name=f"sc{asfx}",
                                                tag="bc", bufs=1)
                            binv = work.tile([P, 512], F32, name=f"binv{asfx}",
                                             tag="binv", bufs=1)
                            nc.vector.reciprocal_approx_accurate(binv, pbc, scratch)
                            nc.vector.tensor_tensor(
                                hT[:, h, qc * 512:(qc + 1) * 512], pav, binv,
                                mybir.AluOpType.mult)

                        return emitA, emitB

                    # qc-major so all heads' qc=0 results complete before the
                    # qc=1 ones — lets the last batch's phase3 rows 0:512
                    # start while qc=1 attention is still draining
                    for qc in range(NC2):
                        for hh in range(GH):
                            attn_queue.append(make_attn(g, hh, qc))

                    # previous batch's output projection: emitted here (after
                    # this group's proj + attn queueing) so its po matmuls
                    # interleave with this group's attention units
                    if pending:
                        for t in pending:
                            t()
                        pending = []

                # phase 3 of this batch is deferred: it is emitted after the
                # next batch's first projection group so its matmuls overlap
                # the last attention units
                pending = [make_phase3_half(b, hT, 0), make_phase3_half(b, hT, 1)]

            # tail: queue holds the last group's 4 attention units (qc-major).
            # Drain qc=0 so all heads' rows 0:512 are complete, then phase3
            # half 0 (pops the qc=1 units between its po matmuls), then half 1.
            pop_a()   # h6 qc0
            pop_a()   # h7 qc0
            pop_b()
            pop_b()
            emit_h0, emit_h1 = pending
            emit_h0()
            while attn_queue or b_queue:
                pop_a()
                pop_b()
            emit_h1()

    nc.compile()
    return nc


_NC_CACHE = {}


def _get_nc(has_bias):
    if has_bias not in _NC_CACHE:
        _NC_CACHE[has_bias] = build_nc(has_bias)
    return _NC_CACHE[has_bias]


def _pack_w3(W):
    """[D, D] -> [P, G, DB, GW] with W[p,g,db,e] = W[db*128+p, g*256+e]."""
    a = np.asarray(W, dtype=np.float32).reshape(DB, P, G, GW)
    return np.ascontiguousarray(a.transpose(1, 2, 0, 3)).astype(ml_dtypes.bfloat16)


def make_in_maps(x, Wq, bq, Wk, bk, Wv, bv, Wo):
    bf16 = ml_dtypes.bfloat16
    x = np.asarray(x, dtype=np.float32)
    wo = np.asarray(Wo, dtype=np.float32).reshape(DB, P, D)
    shared = {
        "Wq": _pack_w3(Wq),
        "Wk": _pack_w3(Wk),
        "Wv": _pack_w3(Wv),
        "Wo": np.ascontiguousarray(wo.transpose(1, 0, 2)).astype(bf16),
        "bq": np.ascontiguousarray(bq, dtype=np.float32),
        "bk": np.ascontiguousarray(bk, dtype=np.float32),
        "bv": np.ascontiguousarray(bv, dtype=np.float32).astype(bf16),
    }
    in_maps = []
    for c in range(N_CORES):
        xc = x[c * BPC:(c + 1) * BPC]                 # [BPC, N, D]
        # [b, p, db, n] = x[b, n, db*128+p]
        xTc = xc.transpose(0, 2, 1).reshape(BPC, DB, P, N).transpose(0, 2, 1, 3)
        in_maps.append({"xT": np.ascontiguousarray(xTc).astype(bf16), **shared})
    return in_maps


def run(x, Wq, bq, Wk, bk, Wv, bv, Wo, trace=False):
    has_bias = bool(np.any(np.asarray(bq)) or np.any(np.asarray(bk))
                    or np.any(np.asarray(bv)))
    nc = _get_nc(has_bias)
    in_maps = make_in_maps(x, Wq, bq, Wk, bk, Wv, bv, Wo)
    res = run_bass_kernel_spmd(nc, in_maps, list(range(N_CORES)), trace=trace)
    # out[b, p, nb, d] -> full[b, nb*128+p, d]
    out = np.concatenate(
        [np.asarray(res.results[c]["out"]).transpose(0, 2, 1, 3).reshape(BPC, N, D)
         for c in range(N_CORES)], axis=0).astype(np.float32)
    return out, res


def kernel(x, Wq, bq, Wk, bk, Wv, bv, Wo):
    out, _ = run(x, Wq, bq, Wk, bk, Wv, bv, Wo, trace=False)
    return out
